# revision 1
# baseline (speedup 1.0000x reference)
"""Trainium2 Bass kernel for nn_Codec (5-level lifting wavelet codec stats).

kernel(**inputs) takes the FULL inputs (x [32,3,512,512] f32 + eight 3-tap
filters) and returns (loss1, loss0, invCR0, invCR1) as np.float32 scalars.

Sharding: pure data parallel - 96 (batch*channel) slices split 12 per core
across 8 NeuronCores; scalar partials are gathered and reduced on the host.

Per-slice device pipeline (v4):
  - Input slices shipped bf16 (host-cast, halves the HBM load).
  - Lifting levels 0-1 entirely on TensorE as bf16 banded matmuls against
    host-composed matrices, with bf16 DMA-xbar transposes between the x and
    y phases. Levels 2-4 (tiny) keep PE x-matmuls + DVE y-convs, all bf16.
  - Subbands land in a [128, 2048] bf16 staging tile (orientation-free for
    the histogram); deep sub-128-partition bands placed by small DMAs.
  - Integer-direct binning: bi = rne(v*128 + 127.5+2^-10) == 128+floor(128v)
    exactly for bf16 v; bin = bi & 255 via shift/mask; invalid (v < -1)
    elements pushed out of the l one-hot range (l += 16).
  - Radix-16x16 (delta) and 16x8 (img) one-hot bf16 mask planes at FC=1024,
    written by DVE (4x mode) with a tunable share on GPSIMD (which supports
    tensor_scalar is_equal); joint counts via TensorE into PSUM.
  - RMSE sums via ScalarE Square+accum (split slots per half-chunk); the
    fmod-vs-posmod correction via affine_mul_reduce.
"""

import os

import numpy as np
from contextlib import ExitStack

import concourse.bass as bass
import concourse.mybir as mybir
import concourse.tile as tile
from concourse import bacc
from concourse.bass_utils import run_bass_kernel_spmd

F32 = mybir.dt.float32
BF16 = mybir.dt.bfloat16
I32 = mybir.dt.int32
ALU = mybir.AluOpType
ACTF = mybir.ActivationFunctionType

N_CORES = 8
S0 = 512
NSL = 12            # slices per core (96 / 8)
STG = 2048          # staging free dim per slice (512*512/128)
RES = S0 * S0
FC = int(os.environ.get("KB_FC", "1024"))  # mask chunk width (free dim)
HC = 1024           # prep sub-chunk width
N_LEVELS = 5
G = 8               # sub-chunk dup factor for the joint-count matmuls
C0 = 127.5009765625  # 127.5 + 2^-10: rne(v*128 + C0) == 128 + floor(v*128)

# tap vector layout (y-phase DVE convs for levels 2-4 only).
TP_RY, TP_NPY, TP_NCY = 0, 3, 6
NT = 9

# ---------------------------------------------------------------------------
# host-side matrix composition + block plans
# ---------------------------------------------------------------------------


def _make_mats(S, p, u, c, r, update):
    """A (odd out) and B (even out) lifting matrices [S/2, S], f64->f32.
    update=False omits the update step (y-lifting at levels >= 2)."""
    half = S // 2
    E = np.zeros((half, S))
    O = np.zeros((half, S))
    E[np.arange(half), 2 * np.arange(half)] = 1.0
    O[np.arange(half), 2 * np.arange(half) + 1] = 1.0

    def T(k):
        M = np.zeros((half, half))
        i = np.arange(half)
        M[i, i] = k[1]
        M[i[1:], i[1:] - 1] = k[0]
        M[i[:-1], i[:-1] + 1] = k[2]
        return M

    o1 = O - T(p.astype(np.float64)) @ E
    e1 = E + T(u.astype(np.float64)) @ o1 if update else E
    A = o1 - T(c.astype(np.float64)) @ e1
    B = e1 + T(r.astype(np.float64)) @ A
    return A.astype(np.float32), B.astype(np.float32)


def _plan(S):
    """Static nonzero-block structure for a [S/2, S] composed lifting matrix
    (band halfwidth <= 10 in the S domain): per out-tile r, the list of
    128-wide K-block cols that are structurally nonzero."""
    half = S // 2
    t_out = max(1, (half + 127) // 128)
    t_in = max(1, (S + 127) // 128)
    rows = []
    for r in range(t_out):
        m0 = 128 * r
        m1 = min(m0 + 128, half)
        j0 = max(0, 2 * m0 - 10)
        j1 = min(S - 1, 2 * (m1 - 1) + 10)
        rows.append([c for c in range(t_in) if 128 * c <= j1 and 128 * c + 127 >= j0])
    return rows


X_PLANS = [_plan(S0 >> lvl) for lvl in range(N_LEVELS)]
Y_PLANS = [_plan(S0 >> lvl) for lvl in range(2)]  # y on PE for lvl 0-1 only

# wyb (bf16) block order: x lvl0 A/B, y lvl0 C/R, x lvl1 A/B, y lvl1 C/R,
# x lvl2 A/B, x lvl3 A/B, x lvl4 A/B
NWB = (
    2 * sum(len(cs) for cs in X_PLANS[0])
    + 2 * sum(len(cs) for cs in Y_PLANS[0])
    + 2 * sum(len(cs) for cs in X_PLANS[1])
    + 2 * sum(len(cs) for cs in Y_PLANS[1])
    + 2 * sum(len(cs) for cs in X_PLANS[2])
    + 2 * sum(len(cs) for cs in X_PLANS[3])
    + 2 * sum(len(cs) for cs in X_PLANS[4])
)


def _pack_blocks(M_, plan, buf, i):
    half, S = M_.shape[0], M_.shape[1]
    for r, cs in enumerate(plan):
        m0, m1 = 128 * r, min(128 * r + 128, half)
        for c in cs:
            k0, k1 = 128 * c, min(128 * c + 128, S)
            buf[i, : k1 - k0, : m1 - m0] = M_[m0:m1, k0:k1].T
            i += 1
    return i


def _check_cover(M_, plan):
    half, S = M_.shape
    mass = np.abs(M_).sum()
    cov = 0.0
    for r, cs in enumerate(plan):
        m0, m1 = 128 * r, min(128 * r + 128, half)
        for c in cs:
            k0, k1 = 128 * c, min(128 * c + 128, S)
            cov += np.abs(M_[m0:m1, k0:k1]).sum()
    assert abs(cov - mass) < 1e-6 * max(mass, 1), (half, S, cov, mass)


def _build_w_host(px, ux, cx, rx, py, uy, cy, ry):
    xmats = [_make_mats(S0 >> l, px, ux, cx, rx, update=True) for l in range(N_LEVELS)]
    ymats = [_make_mats(S0 >> l, py, uy, cy, ry, update=(l < 2)) for l in range(2)]
    for lvl in range(N_LEVELS):
        for M_ in xmats[lvl]:
            _check_cover(M_, X_PLANS[lvl])
    for lvl in range(2):
        for M_ in ymats[lvl]:
            _check_cover(M_, Y_PLANS[lvl])

    wyb = np.zeros((NWB, 128, 128), np.float32)
    i = 0
    for M_ in xmats[0]:
        i = _pack_blocks(M_, X_PLANS[0], wyb, i)
    for M_ in ymats[0]:
        i = _pack_blocks(M_, Y_PLANS[0], wyb, i)
    for M_ in xmats[1]:
        i = _pack_blocks(M_, X_PLANS[1], wyb, i)
    for M_ in ymats[1]:
        i = _pack_blocks(M_, Y_PLANS[1], wyb, i)
    for lvl in (2, 3, 4):
        for M_ in xmats[lvl]:
            i = _pack_blocks(M_, X_PLANS[lvl], wyb, i)
    assert i == NWB, (i, NWB)
    return wyb


# staging slots for the deep subbands (levels 2-4). (p0, p1, c0, c1)
DEEP_SLOTS = {
    "l2xo2": (0, 64, 1920, 2048),
    "l2yo2": (64, 128, 1920, 1984),
    "l3xo2": (64, 96, 1984, 2048),
    "l3yo2": (96, 128, 1984, 2016),
    "l4xo2": (96, 112, 2016, 2048),
    "l4yo2": (112, 128, 2016, 2032),
    "ye4": (112, 128, 2032, 2048),
}

# elementwise prep routing: "v" = DVE, "g" = GPSIMD (tensor_scalar-compatible
# ops only: converts/compares; int bit-ops and STT must stay on DVE)
ROUTE = {"bi_d": "a", "bi_i": "a"}
for _k in os.environ.get("KB_ROUTE_V", "").split(","):
    if _k:
        ROUTE.pop(_k, None)
for _k in os.environ.get("KB_ROUTE_G", "").split(","):
    if _k:
        ROUTE[_k] = "g"

YPSUM_BUFS = int(os.environ.get("KB_YPSUM_BUFS", "3"))
MASK_BUFS = int(os.environ.get("KB_MASK_BUFS", "2"))
DIGIT_BUFS = int(os.environ.get("KB_DIGIT_BUFS", "1"))
LIFT_BUFS = int(os.environ.get("KB_LIFT_BUFS", "2"))
ABL_LIFT = os.environ.get("KB_ABL_LIFT", "1") == "1"
ABL_HIST = os.environ.get("KB_ABL_HIST", "1") == "1"
ABL_PREP = os.environ.get("KB_ABL_PREP", "1") == "1"
ABL_MM = os.environ.get("KB_ABL_MM", "1") == "1"
ABL_ACC = os.environ.get("KB_ABL_ACC", "1") == "1"
POOL_PLANES_DH = int(os.environ.get("KB_POOL_DH", "4"))
POOL_PLANES_IH = int(os.environ.get("KB_POOL_IH", "4"))
ACT_PLANES_DH = int(os.environ.get("KB_ACT_DH", "1"))
ACT_PLANES_IH = int(os.environ.get("KB_ACT_IH", "1"))
POOL_PLANES_DL = int(os.environ.get("KB_POOL_DL", "0"))
POOL_PLANES_IL = int(os.environ.get("KB_POOL_IL", "0"))

# ---------------------------------------------------------------------------
# device kernel
# ---------------------------------------------------------------------------


def _conv_step(nc, out_ap, base_ap, src_ap, tap_col, tp_sb, P, F):
    """out = base + 3-tap conv of src along the free dim, zero padding."""
    k0 = tp_sb[0:P, tap_col : tap_col + 1]
    k1 = tp_sb[0:P, tap_col + 1 : tap_col + 2]
    k2 = tp_sb[0:P, tap_col + 2 : tap_col + 3]
    nc.vector.scalar_tensor_tensor(out_ap, src_ap, k1, base_ap, ALU.mult, ALU.add)
    nc.vector.scalar_tensor_tensor(
        out_ap[:, 1:F], src_ap[:, 0 : F - 1], k0, out_ap[:, 1:F], ALU.mult, ALU.add
    )
    nc.vector.scalar_tensor_tensor(
        out_ap[:, 0 : F - 1], src_ap[:, 1:F], k2, out_ap[:, 0 : F - 1], ALU.mult, ALU.add
    )


def _hist_pipeline(nc, pools, hb_srcs, lb_srcs, n_h, h0, n_l, psum_ap, pool_h,
                   act_h=0, pool_l=0):
    """One-hot mask + joint-count matmuls over [128, STG] bf16 digit sources.
    h-planes [0, pool_h) -> GPSIMD; [pool_h, pool_h+act_h) -> ScalarE via
    Relu(1-|h-a|) (exact for integer digits); the rest -> DVE is_equal."""
    mpool = pools["masks"]
    sc = pools["scratch"]
    bias_negs = pools["bias_negs"]
    n_chunks = STG // FC
    n_mm = FC // G
    for ch in range(n_chunks):
        t0, o0 = (ch * FC) // HC, (ch * FC) % HC
        hb_flat = hb_srcs[t0][:, o0 : o0 + FC]
        hb_src = hb_flat.rearrange("p (n g) -> p n g", g=G)
        lb_src = lb_srcs[t0][:, o0 : o0 + FC].rearrange("p (n g) -> p n g", g=G)
        mh = mpool.tile([128, n_mm, 16 * G], BF16, tag="mh")
        ml = mpool.tile([128, n_mm, 16 * G], BF16, tag="ml")
        for a in range(n_h):
            if pool_h <= a < pool_h + act_h:
                at = sc.tile([128, FC], BF16, tag="AT")
                nc.scalar.activation(
                    at[:], hb_flat, ACTF.Abs,
                    bias=bias_negs[:, h0 + a : h0 + a + 1],
                )
                nc.scalar.activation(
                    mh[:, :, a * G : (a + 1) * G], at[:], ACTF.Relu,
                    bias=bias_negs[:, 32:33], scale=-1.0,
                )
                continue
            e = nc.gpsimd if a < pool_h else nc.vector
            e.tensor_scalar(
                mh[:, :, a * G : (a + 1) * G],
                hb_src,
                float(h0 + a),
                None,
                ALU.is_equal,
            )
        for b in range(n_l):
            el = nc.gpsimd if b < pool_l else nc.vector
            el.tensor_scalar(
                ml[:, :, b * G : (b + 1) * G],
                lb_src,
                float(b),
                None,
                ALU.is_equal,
            )
        for g_ in range(n_mm):
            nc.tensor.matmul(
                psum_ap,
                mh[:, g_, 0 : n_h * G],
                ml[:, g_, 0 : n_l * G],
                start=(ch == 0 and g_ == 0),
                stop=(ch == n_chunks - 1 and g_ == n_mm - 1),
                skip_group_check=True,
            )


def build_nc(nsl=NSL):
    PE_TPOSE = os.environ.get("KB_PE_TPOSE", "1") == "1"

    nc = bacc.Bacc("TRN2", target_bir_lowering=False, debug=False)
    xs = nc.dram_tensor("xs", [nsl, S0, S0], BF16, kind="ExternalInput")
    tp = nc.dram_tensor("tp", [NT], F32, kind="ExternalInput")
    wyb = nc.dram_tensor("wyb", [NWB, 128, 128], BF16, kind="ExternalInput")
    idn = nc.dram_tensor("idn", [128, 128], BF16, kind="ExternalInput")
    pd = nc.dram_tensor("pd", [nsl, 128, 128], F32, kind="ExternalOutput")
    pi = nc.dram_tensor("pi", [nsl, 128, 64], F32, kind="ExternalOutput")
    accd = nc.dram_tensor("accd", [128, nsl * 8], F32, kind="ExternalOutput")

    def V(name):
        return nc.gpsimd if ROUTE.get(name) == "g" else nc.vector

    with tile.TileContext(nc) as tc:
        with ExitStack() as ctx:
            const = ctx.enter_context(tc.tile_pool(name="const", bufs=1))
            xpool = ctx.enter_context(tc.tile_pool(name="xpool", bufs=2))
            stgp = ctx.enter_context(tc.tile_pool(name="stgp", bufs=2))
            lift = ctx.enter_context(tc.tile_pool(name="lift", bufs=LIFT_BUFS))
            work = ctx.enter_context(tc.tile_pool(name="work", bufs=2))
            liftb = ctx.enter_context(tc.tile_pool(name="liftb", bufs=1))
            scratch = ctx.enter_context(tc.tile_pool(name="scratch", bufs=1))
            maskp = ctx.enter_context(tc.tile_pool(name="masks", bufs=MASK_BUFS))
            psum = ctx.enter_context(tc.tile_pool(name="psum", bufs=int(os.environ.get("KB_PSUM_BUFS", "2")), space="PSUM"))
            ypsum = ctx.enter_context(tc.tile_pool(name="ypsum", bufs=YPSUM_BUFS, space="PSUM"))

            tp_sb = const.tile([128, NT], F32)
            nc.sync.dma_start(
                tp_sb[:], tp.ap().rearrange("(o n) -> o n", o=1).broadcast_to([128, NT])
            )
            wyb_sb = const.tile([128, NWB * 128], BF16)
            nc.sync.dma_start(
                wyb_sb[:].rearrange("k (n m) -> k n m", n=NWB),
                wyb.ap().rearrange("n k m -> k n m"),
            )
            idn_sb = const.tile([128, 128], BF16, tag="idn")
            nc.sync.dma_start(idn_sb[:], idn.ap())
            acc = const.tile([128, nsl * 8], F32)
            nc.vector.memset(acc[:], 0.0)
            biasC0 = const.tile([128, 1], F32, tag="biasC0")
            nc.vector.memset(biasC0[:], C0)
            bias_negs = const.tile([128, 33], F32, tag="bias_negs")
            for _k in range(32):
                nc.vector.memset(bias_negs[:, _k : _k + 1], -float(_k))
            nc.vector.memset(bias_negs[:, 32:33], 1.0)
            pools = {"masks": maskp, "scratch": scratch,
                     "bias_negs": bias_negs}

            nb_x0 = 2 * sum(len(cs) for cs in X_PLANS[0])
            nb_y0 = nb_x0 + 2 * sum(len(cs) for cs in Y_PLANS[0])
            nb_x1 = nb_y0 + 2 * sum(len(cs) for cs in X_PLANS[1])
            nb_y1 = nb_x1 + 2 * sum(len(cs) for cs in Y_PLANS[1])

            for s in range(nsl):
                # ---- load slice (transposed, bf16): x_sb[p, t, h]
                x_sb = xpool.tile([128, 4, S0], BF16, tag="x_sb")
                nc.sync.dma_start(
                    x_sb[:], xs.ap()[s].rearrange("(t p) w -> p t w", p=128)
                )
                x_flat = x_sb[:].rearrange("p t w -> p (t w)")

                stg = stgp.tile([128, STG], BF16, tag="stg")
                if not ABL_LIFT:
                    nc.vector.memset(stg[:, 0:1], 0.3)
                # ================= level 0 (PE, bf16) =====================
                if ABL_LIFT:
                    xe2b = liftb.tile([128, 2, S0], BF16, tag="xe2b0")
                    wf = 0
                    for kind in range(2):  # 0: A (xo), 1: B (xe2)
                        for r, cs in enumerate(X_PLANS[0]):
                            ps = ypsum.tile([128, 512], F32, tag="yps")
                            for i, c in enumerate(cs):
                                nc.tensor.matmul(
                                    ps[:, :],
                                    wyb_sb[0:128, 128 * wf : 128 * wf + 128],
                                    x_sb[:, c, :],
                                    start=(i == 0),
                                    stop=(i == len(cs) - 1),
                                    skip_group_check=True,
                                )
                                wf += 1
                            if kind == 0:
                                nc.scalar.copy(stg[:, 512 * r : 512 * (r + 1)], ps[:, :])
                            else:
                                nc.scalar.copy(xe2b[:, r, :], ps[:, :])
                    assert wf == nb_x0

                    # transpose xe2 [256, 512] -> xe2T (bf16 xbar)
                    xe2T = lift.tile([128, 2, 4, 128], BF16, tag="xe2T0")
                    for r in range(2):
                        nc.sync.dma_start_transpose(xe2T[:, r, :, :], xe2b[:, r, :])

                    # y-phase: yo = C0 @ xe2T, ye2 = R0 @ xe2T, per (r2, r)
                    ye2b = liftb.tile([128, 2, 2, 128], BF16, tag="ye2b0")
                    wb = nb_x0
                    for kind in range(2):  # 0: C (yo), 1: R (ye2)
                        for r2, cs in enumerate(Y_PLANS[0]):
                            blk0 = wb
                            for r in range(2):
                                ps2f = ypsum.tile([128, 512], F32, tag="yps")
                                ps2 = ps2f[:, 0:128]
                                for i, c2 in enumerate(cs):
                                    nc.tensor.matmul(
                                        ps2,
                                        wyb_sb[0:128, 128 * (blk0 + i) : 128 * (blk0 + i) + 128],
                                        xe2T[:, r, c2, :],
                                        start=(i == 0),
                                        stop=(i == len(cs) - 1),
                                        skip_group_check=True,
                                    )
                                if kind == 0:
                                    col = 1024 + 128 * (2 * r2 + r)
                                    nc.scalar.copy(stg[:, col : col + 128], ps2)
                                else:
                                    nc.scalar.copy(ye2b[:, r2, r, :], ps2)
                            wb = blk0 + len(cs)
                    assert wb == nb_y0

                    cur1 = lift.tile([128, 2, 256], BF16, tag="cur1")
                    for r2 in range(2):
                        for r in range(2):
                            nc.sync.dma_start_transpose(
                                cur1[:, r, 128 * r2 : 128 * r2 + 128], ye2b[:, r2, r, :]
                            )

                    # ================= level 1 (PE bf16) ======================
                    xe2b1 = liftb.tile([128, 256], BF16, tag="xe2b1")
                    for kind in range(2):
                        base = nb_y0 + (0 if kind == 0 else len(X_PLANS[1][0]))
                        psf = ypsum.tile([128, 512], F32, tag="yps")
                        ps = psf[:, 0:256]
                        cs = X_PLANS[1][0]
                        for i, c in enumerate(cs):
                            nc.tensor.matmul(
                                ps,
                                wyb_sb[0:128, 128 * (base + i) : 128 * (base + i) + 128],
                                cur1[:, c, :],
                                start=(i == 0),
                                stop=(i == len(cs) - 1),
                                skip_group_check=True,
                            )
                        if kind == 0:
                            nc.scalar.copy(stg[:, 1536:1792], ps)
                        else:
                            nc.scalar.copy(xe2b1[:], ps)

                    xe2T1 = lift.tile([128, 2, 128], BF16, tag="xe2T1")
                    nc.sync.dma_start_transpose(xe2T1[:, :, :], xe2b1[:])

                    ye2b1 = liftb.tile([128, 128], BF16, tag="ye2b1")
                    for kind in range(2):
                        base = nb_x1 + (0 if kind == 0 else len(Y_PLANS[1][0]))
                        ps2f = ypsum.tile([128, 512], F32, tag="yps")
                        ps2 = ps2f[:, 0:128]
                        cs = Y_PLANS[1][0]
                        for i, c2 in enumerate(cs):
                            nc.tensor.matmul(
                                ps2,
                                wyb_sb[0:128, 128 * (base + i) : 128 * (base + i) + 128],
                                xe2T1[:, c2, :],
                                start=(i == 0),
                                stop=(i == len(cs) - 1),
                                skip_group_check=True,
                            )
                        if kind == 0:
                            nc.scalar.copy(stg[:, 1792:1920], ps2)
                        else:
                            nc.scalar.copy(ye2b1[:], ps2)

                    cur2 = lift.tile([128, 128], BF16, tag="cur2")
                    nc.sync.dma_start_transpose(cur2[:, :], ye2b1[:])

                    # ================= levels 2-4 (PE x bf16, DVE y) ==========
                    cur_tiles = [(cur2[:, :], 128)]
                    wb2 = nb_y1
                    for lvl in range(2, N_LEVELS):
                        S = S0 >> lvl
                        half = S // 2
                        xe2_tiles = []
                        for kind in range(2):
                            cs = X_PLANS[lvl][0]
                            M = half
                            psf = ypsum.tile([128, 512], F32, tag="yps")
                            ps = psf[0:M, 0:S]
                            for i, c in enumerate(cs):
                                ap, K = cur_tiles[c]
                                nc.tensor.matmul(
                                    ps,
                                    wyb_sb[0:K, 128 * wb2 : 128 * wb2 + M],
                                    ap,
                                    start=(i == 0),
                                    stop=(i == len(cs) - 1),
                                    skip_group_check=True,
                                )
                                wb2 += 1
                            if kind == 0:
                                if lvl == 2:
                                    p0, p1, q0, q1 = DEEP_SLOTS["l2xo2"]
                                    nc.scalar.copy(stg[p0:p1, q0:q1], ps)
                                else:
                                    key = "l3xo2" if lvl == 3 else "l4xo2"
                                    p0, p1, q0, q1 = DEEP_SLOTS[key]
                                    xo2s = work.tile([M, S], BF16, tag=f"xo2s_{lvl}")
                                    nc.scalar.copy(xo2s[:], ps)
                                    nc.sync.dma_start(stg[p0:p1, q0:q1], xo2s[:])
                            else:
                                xe2 = work.tile([M, S], BF16, tag=f"xe2_{lvl}")
                                nc.scalar.copy(xe2[:], ps)
                                xe2_tiles.append((xe2, M))

                        new_cur = []
                        for xe2, P in xe2_tiles:
                            ye_v = xe2[0:P, 0:S:2]
                            yo_v = xe2[0:P, 1:S:2]
                            yo1 = work.tile([P, half], BF16, tag=f"yo1_{lvl}")
                            _conv_step(nc, yo1[:], yo_v, ye_v, TP_NPY, tp_sb, P, half)
                            yo2_t = work.tile([P, half], BF16, tag=f"yo2_{lvl}")
                            _conv_step(nc, yo2_t[:], yo1[:], ye_v, TP_NCY, tp_sb, P, half)
                            key = {2: "l2yo2", 3: "l3yo2", 4: "l4yo2"}[lvl]
                            p0, p1, q0, q1 = DEEP_SLOTS[key]
                            nc.sync.dma_start(stg[p0:p1, q0:q1], yo2_t[:])
                            ye2 = work.tile([P, half], BF16, tag=f"ye2_{lvl}")
                            _conv_step(nc, ye2[:], ye_v, yo2_t[:], TP_RY, tp_sb, P, half)
                            if lvl < N_LEVELS - 1:
                                new_cur.append((ye2[:], P))
                            else:
                                p0, p1, q0, q1 = DEEP_SLOTS["ye4"]
                                nc.sync.dma_start(stg[p0:p1, q0:q1], ye2[:])
                        cur_tiles = new_cur

                # ================= binning prep ===========================
                sc = scratch
                NH = STG // HC  # prep sub-chunks

                # ---- img (x bf16): bin = bi in [128, 256]; h=bi>>3, l=bi&7
                lb_is, hb_is = [], []
                for h in range(NH):
                    lo, hi = h * HC, (h + 1) * HC
                    lb_ih = sc.tile([128, HC], BF16, tag=f"lb_i{h}")
                    hb_ih = sc.tile([128, HC], BF16, tag=f"hb_i{h}")
                    lb_is.append(lb_ih)
                    hb_is.append(hb_ih)
                    bi_i = sc.tile([128, HC], I32, tag="A")
                    if ROUTE.get("bi_i") == "a":
                        nc.scalar.activation(
                            bi_i[:], x_flat[:, lo:hi], ACTF.Identity,
                            bias=biasC0[:, 0:1], scale=128.0,
                        )
                    else:
                        V("bi_i").tensor_scalar(
                            bi_i[:], x_flat[:, lo:hi], 128.0, C0, ALU.mult, ALU.add
                        )
                    l_ii = sc.tile([128, HC], I32, tag="B")
                    nc.vector.tensor_scalar(l_ii[:], bi_i[:], 7, None, ALU.bitwise_and)
                    h_ii = sc.tile([128, HC], I32, tag="C")
                    nc.vector.tensor_scalar(
                        h_ii[:], bi_i[:], 3, None, ALU.arith_shift_right
                    )
                    nc.scalar.activation(lb_ih[:], l_ii[:], ACTF.Identity)
                    nc.scalar.activation(hb_ih[:], h_ii[:], ACTF.Identity)
                    if ABL_ACC:
                        junk_i = sc.tile([128, HC], BF16, tag="J")
                        nc.scalar.activation(
                            junk_i[:], x_flat[:, lo:hi], ACTF.Square,
                            accum_out=acc[:, s * 8 + 4 + h : s * 8 + 5 + h],
                        )

                ps_if = psum.tile([128, 128], F32, tag="ps")
                ps_i = ps_if[:, 0:64]
                if ABL_HIST:
                    _hist_pipeline(
                        nc, pools, [t[:] for t in hb_is], [t[:] for t in lb_is],
                        16, 16, 8, ps_i, POOL_PLANES_IH, act_h=ACT_PLANES_IH,
                        pool_l=POOL_PLANES_IL,
                    )
                else:
                    nc.tensor.matmul(ps_i, lb_is[0][:, 0:128], lb_is[0][:, 0:64], start=True, stop=True, skip_group_check=True)
                pi_sb = work.tile([128, 64], F32, tag="pi_sb")
                nc.scalar.copy(pi_sb[:], ps_i)
                nc.sync.dma_start(pi.ap()[s], pi_sb[:])

                # ---- delta (stg bf16)
                lxs, hb_ds = [], []
                for h in range(NH):
                    lo, hi = h * HC, (h + 1) * HC
                    lxh = sc.tile([128, HC], BF16, tag=f"lx{h}")
                    hb_dh = sc.tile([128, HC], BF16, tag=f"hb_d{h}")
                    lxs.append(lxh)
                    hb_ds.append(hb_dh)
                    bi_d = sc.tile([128, HC], I32, tag="A")
                    if ROUTE.get("bi_d") == "a":
                        nc.scalar.activation(
                            bi_d[:], stg[:, lo:hi], ACTF.Identity,
                            bias=biasC0[:, 0:1], scale=128.0,
                        )
                    else:
                        V("bi_d").tensor_scalar(
                            bi_d[:], stg[:, lo:hi], 128.0, C0, ALU.mult, ALU.add
                        )
                    l_d = sc.tile([128, HC], I32, tag="B")
                    nc.vector.tensor_scalar(l_d[:], bi_d[:], 15, None, ALU.bitwise_and)
                    h_d = sc.tile([128, HC], I32, tag="C")
                    nc.vector.tensor_scalar(
                        h_d[:], bi_d[:], 4, 15, ALU.arith_shift_right, ALU.bitwise_and
                    )
                    lb_d = sc.tile([128, HC], BF16, tag="D")
                    nc.scalar.activation(lb_d[:], l_d[:], ACTF.Identity)
                    nc.scalar.activation(hb_dh[:], h_d[:], ACTF.Identity)
                    cng = sc.tile([128, HC], BF16, tag="E")
                    V("cng").tensor_scalar(cng[:], stg[:, lo:hi], -1.0, None, ALU.is_lt)
                    nc.vector.scalar_tensor_tensor(
                        lxh[:], cng[:], 16.0, lb_d[:], ALU.mult, ALU.add
                    )
                    fl = sc.tile([128, HC], I32, tag="B")
                    nc.vector.tensor_scalar(
                        fl[:], bi_d[:], 8, None, ALU.arith_shift_right
                    )
                    dl = sc.tile([128, HC], F32, tag="C")
                    if ROUTE.get("dl") == "g":
                        flf = sc.tile([128, HC], F32, tag="FF")
                        nc.scalar.activation(flf[:], fl[:], ACTF.Identity, scale=-2.0)
                        nc.gpsimd.tensor_tensor(dl[:], flf[:], stg[:, lo:hi], ALU.add)
                    else:
                        nc.vector.scalar_tensor_tensor(
                            dl[:], fl[:], -2.0, stg[:, lo:hi], ALU.mult, ALU.add
                        )
                    if ABL_ACC:
                        junk_d = sc.tile([128, HC], BF16, tag="J")
                        nc.scalar.activation(
                            junk_d[:], dl[:], ACTF.Square,
                            accum_out=acc[:, s * 8 + h : s * 8 + 1 + h],
                        )
                        junk_d2 = sc.tile([128, HC], BF16, tag="J")
                        nc.vector.affine_mul_reduce(
                            junk_d2[:], acc[:, s * 8 + 2 + h : s * 8 + 3 + h],
                            dl[:], cng[:], -4.0, 4.0,
                        )

                ps_d = psum.tile([128, 128], F32, tag="ps")
                if ABL_HIST:
                    _hist_pipeline(
                        nc, pools, [t[:] for t in hb_ds], [t[:] for t in lxs],
                        16, 0, 16, ps_d[:], POOL_PLANES_DH, act_h=ACT_PLANES_DH,
                        pool_l=POOL_PLANES_DL,
                    )
                else:
                    nc.tensor.matmul(ps_d[:], lxs[0][:, 0:128], lxs[0][:, 0:128], start=True, stop=True, skip_group_check=True)
                pd_sb = work.tile([128, 128], F32, tag="pd_sb")
                nc.scalar.copy(pd_sb[:], ps_d[:])
                nc.sync.dma_start(pd.ap()[s], pd_sb[:])

            nc.sync.dma_start(accd.ap()[:, :], acc[:])

    nc.compile()
    return nc


_NC_CACHE = {}


def _get_nc():
    if "nc" not in _NC_CACHE:
        _NC_CACHE["nc"] = build_nc()
    return _NC_CACHE["nc"]


LAST_INFO = {}


def kernel(x, px, ux, cx, rx, py, uy, cy, ry, _trace=False):
    import ml_dtypes

    x = np.asarray(x, dtype=np.float32)
    px, ux, cx, rx, py, uy, cy, ry = (
        np.asarray(k, dtype=np.float32) for k in (px, ux, cx, rx, py, uy, cy, ry)
    )

    nc = _get_nc()

    tp_host = np.zeros(NT, np.float32)
    tp_host[TP_RY : TP_RY + 3] = ry
    tp_host[TP_NPY : TP_NPY + 3] = -py
    tp_host[TP_NCY : TP_NCY + 3] = -cy
    wyb_host = _build_w_host(px, ux, cx, rx, py, uy, cy, ry).astype(ml_dtypes.bfloat16)

    # W-major (transposed) slices, cast bf16 on the host
    shards = np.ascontiguousarray(
        x.reshape(N_CORES, NSL, S0, S0).transpose(0, 1, 3, 2)
    ).astype(ml_dtypes.bfloat16)
    idn_host = np.eye(128, dtype=ml_dtypes.bfloat16)
    in_maps = [
        {"xs": np.ascontiguousarray(shards[i]), "tp": tp_host, "wyb": wyb_host,
         "idn": idn_host}
        for i in range(N_CORES)
    ]
    if not _trace:
        os.environ.setdefault("BASS_NEVER_TRACE", "1")
    res = run_bass_kernel_spmd(nc, in_maps, core_ids=list(range(N_CORES)), trace=_trace)
    LAST_INFO["exec_time_ns"] = res.exec_time_ns
    LAST_INFO["results"] = res

    counts_img = np.zeros((96, 256))
    counts_delta = np.zeros((96, 256))
    ss_img = np.zeros(96)
    ss_delta = np.zeros(96)
    for core in range(N_CORES):
        out = res.results[core]
        pd_ = out["pd"].astype(np.float64)
        pi_ = out["pi"].astype(np.float64)
        acc_ = out["accd"].astype(np.float64).sum(axis=0)
        for s in range(NSL):
            gs = core * NSL + s
            cd = np.einsum("afbf->ab", pd_[s].reshape(16, 8, 16, 8)).reshape(256)
            ci = np.einsum("afbf->ab", pi_[s].reshape(16, 8, 8, 8)).reshape(128)
            a = acc_[s * 8 : s * 8 + 8]
            counts_delta[gs] = cd
            counts_img[gs, 128:256] = ci
            ss_delta[gs] = a[0] + a[1] + a[2] + a[3]
            ss_img[gs] = a[4] + a[5]

    loss1 = np.float32(255.0 * np.sqrt(ss_delta.sum() / (96 * RES)))
    loss0 = np.float32(255.0 * np.sqrt(ss_img.sum() / (96 * RES)))

    def ent(counts):
        p = counts / RES
        pz = np.where(p > 0, p, 1.0)
        return float(np.sum(-p * np.log2(pz)))

    invCR0 = np.float32(ent(counts_img) / (8.0 * 96))
    invCR1 = np.float32(ent(counts_delta) / (8.0 * 96))
    LAST_INFO.update(
        counts_img=counts_img, counts_delta=counts_delta, ss_img=ss_img, ss_delta=ss_delta
    )
    return loss1, loss0, invCR0, invCR1

